# revision 8
# baseline (speedup 1.0000x reference)
"""CulturalAttention kernel for 8 TRN2 NeuronCores.

Math: the cultural bias lam * (Q @ c_proj) is constant along the softmax
axis, so softmax(base_logits + bias[..., None]) == softmax(base_logits).
The kernel therefore computes plain attention:
    Q = X@WQ + bQ; K = X@WK + bK; V = X@WV + bV
    out = softmax(Q K^T / 8) V
Logits are ~N(0,1) (WQ/WK pre-scaled by 1/sqrt(D)), so exp() without
max-subtraction is numerically safe in fp32.

Sharding: core = (batch b, query-half h). Each core receives X[b] rotated
so its 2048 queries are rows 0:2047; keys/values use all 4096 rows (softmax
is permutation-invariant in the key axis, so rotation is harmless).

Matmuls run in float32r (single-pass rounded-fp32 on the PE; fp32 proper
would be 4x slower). f32r tiles are produced only by legal rounding
producers: SWDGE cast-DMA, DVE casts, ACT casts.

Per-core dataflow ([s, q] "transposed scores" layout):
  phase 1: cast-DMA X tiles -> PE-transpose to X^T -> K^T/V^T (one packed
           matmul over [WK|WV]), Q^T; V^T is re-transposed to V-natural and
           augmented with a ones column (V') so the AV matmul also produces
           softmax sums.
  phase 2: scores^T[s_tile, q] = K^T_tile^T @ Q^T (PSUM), exp on ScalarE
           (PSUM->SBUF f32r, scale=1/8), O^T[65, q] += V'^T @ expS^T.
  phase 3: O^T -> PE-transpose -> O[q, 65]; col 64 holds the softmax sum;
           multiply cols 0:63 by its reciprocal; DMA out.

This toolchain's walrus encodes at most one semaphore wait per ISA
instruction; _split_waits_json rewrites the BIR json to hoist extra waits
onto single-wait NoOps.
"""

import json
import types

import numpy as np
from contextlib import ExitStack

import concourse.bass as bass
import concourse.tile as tile
from concourse import mybir
from concourse.bass_utils import run_bass_kernel_spmd

B, S, D, DK = 4, 4096, 1024, 64
QC = S // 2          # queries per core
NCORES = 8
F32R = mybir.dt.float32r
F32 = mybir.dt.float32

QB = 1024            # q block in phase 2
NQB = QC // QB       # 2
NT = S // 128        # 32 s-tiles
NBLK = S // 512      # 8 s-blocks in phase 1


def _split_waits_json(bir_bytes: bytes, max_waits: int = 1) -> bytes:
    d = json.loads(bir_bytes)
    cnt = 0
    for f in d["functions"]:
        for bb in f["blocks"]:
            new_insts = []
            for ins in bb["instructions"]:
                si = ins.get("sync_info")
                waits = si.get("on_wait") if si else None
                if waits and len(waits) > max_waits:
                    extra = waits[:-max_waits]
                    si["on_wait"] = waits[-max_waits:]
                    for w in extra:
                        cnt += 1
                        nop = {
                            "engine": ins["engine"], "ins": [], "outs": [],
                            "name": f"{ins['name']}_wsplit{cnt}",
                            "opcode": "NoOp",
                            "sync_info": {"on_update": [], "on_wait": [w]},
                        }
                        if "debug" in ins:
                            nop["debug"] = ins["debug"]
                        new_insts.append(nop)
                new_insts.append(ins)
            bb["instructions"] = new_insts
    return json.dumps(d).encode()


def _install_waitsplit(nc):
    orig = nc.to_json_bytes

    def patched(self):
        return _split_waits_json(orig())

    nc.to_json_bytes = types.MethodType(patched, nc)


def _emit(tc):
    nc = tc.nc
    X_d = nc.declare_dram_parameter("X", [S, D], F32, isOutput=False)
    WQ_d = nc.declare_dram_parameter("WQ", [D, DK], F32, isOutput=False)
    WK_d = nc.declare_dram_parameter("WK", [D, DK], F32, isOutput=False)
    WV_d = nc.declare_dram_parameter("WV", [D, DK], F32, isOutput=False)
    bQ_d = nc.declare_dram_parameter("bQ", [DK, 1], F32, isOutput=False)
    bK_d = nc.declare_dram_parameter("bK", [DK, 1], F32, isOutput=False)
    bV_d = nc.declare_dram_parameter("bV", [DK, 1], F32, isOutput=False)
    I_d = nc.declare_dram_parameter("Ident", [128, 128], F32, isOutput=False)
    O_d = nc.declare_dram_parameter("PadCols", [128, 128 - DK, NT], F32,
                                    isOutput=False)
    Y_d = nc.declare_dram_parameter("Y", [QC, DK], F32, isOutput=True)

    with ExitStack() as ctx:
        consts = ctx.enter_context(tc.tile_pool(name="consts", bufs=1))
        persist = ctx.enter_context(tc.tile_pool(name="persist", bufs=1))

        ident = consts.tile([128, 128], F32R)
        nc.gpsimd.dma_start(out=ident, in_=I_d[:])

        w_kv = consts.tile([128, 8, 128], F32R)
        wq_sb = consts.tile([128, 8, 128], F32R)
        nc.gpsimd.dma_start(out=w_kv[:, :, 0:DK],
                            in_=WK_d[:].rearrange("(c p) k -> p c k", p=128))
        nc.gpsimd.dma_start(out=w_kv[:, :, DK:128],
                            in_=WV_d[:].rearrange("(c p) k -> p c k", p=128))
        nc.gpsimd.dma_start(out=wq_sb[:, :, 0:DK],
                            in_=WQ_d[:].rearrange("(c p) k -> p c k", p=128))
        nc.gpsimd.dma_start(out=wq_sb[:, :, DK:128],
                            in_=WQ_d[:].rearrange("(c p) k -> p c k", p=128))
        bq_sb = consts.tile([DK, 1], F32)
        bk_sb = consts.tile([DK, 1], F32)
        bv_sb = consts.tile([DK, 1], F32)
        nc.sync.dma_start(out=bq_sb, in_=bQ_d[:])
        nc.sync.dma_start(out=bk_sb, in_=bK_d[:])
        nc.sync.dma_start(out=bv_sb, in_=bV_d[:])

        k_sb = persist.tile([DK, S], F32R)             # K^T
        q_sb = persist.tile([DK, QC], F32R)            # Q^T
        # V' = [V | 1 | 0...], st-minor; cols 64.. come from PadCols
        vp_sb = persist.tile([128, 128, NT], F32R)
        nc.gpsimd.dma_start(out=vp_sb[:, DK:128, :], in_=O_d[:])

        # ---------------- phase 1: projections ----------------
        with tc.tile_pool(name="xp", bufs=2) as xp, \
             tc.tile_pool(name="xtp", bufs=2) as xtp, \
             tc.tile_pool(name="vt_st", bufs=2) as vt_stp, \
             tc.tile_pool(name="ps_xt", bufs=2, space="PSUM") as ps_xt, \
             tc.tile_pool(name="ps_kv", bufs=2, space="PSUM") as ps_kv, \
             tc.tile_pool(name="ps_q", bufs=1, space="PSUM") as ps_q, \
             tc.tile_pool(name="ps_v", bufs=1, space="PSUM") as ps_v:
            for blk in range(NBLK):
                x_t = xp.tile([128, 4, D], F32R)   # 4 s-tiles, cast to f32r
                nc.gpsimd.dma_start(
                    out=x_t,
                    in_=X_d[blk * 512:(blk + 1) * 512, :].rearrange(
                        "(j p) d -> p j d", p=128))
                xt_blk = xtp.tile([128, 8, 4, 128], F32R)  # [dm, chunk, j, s]
                for j in range(4):
                    pst = ps_xt.tile([128, 8, 128], F32R)
                    for c in range(8):
                        nc.tensor.transpose(out=pst[:, c, :],
                                            in_=x_t[:, j, c * 128:(c + 1) * 128],
                                            identity=ident)
                    nc.vector.tensor_copy(out=xt_blk[:, :, j, :], in_=pst)

                kv_ps = ps_kv.tile([128, 512], F32)
                for c in range(8):
                    nc.tensor.matmul(out=kv_ps, lhsT=w_kv[:, c, :],
                                     rhs=xt_blk[:, c, :, :],
                                     start=(c == 0), stop=(c == 7))
                sblk = slice(blk * 512, (blk + 1) * 512)
                nc.vector.tensor_scalar_add(out=k_sb[:, sblk],
                                            in0=kv_ps[0:DK, :], scalar1=bk_sb)
                vt_st = vt_stp.tile([DK, 512], F32R)
                nc.vector.tensor_scalar_add(out=vt_st,
                                            in0=kv_ps[DK:128, :], scalar1=bv_sb)
                v_ps = ps_v.tile([128, 4, DK], F32R)
                for j in range(4):
                    nc.tensor.transpose(out=v_ps[:, j, :],
                                        in_=vt_st[:, j * 128:(j + 1) * 128],
                                        identity=ident[0:DK, 0:DK])
                nc.vector.tensor_copy(
                    out=vp_sb[:, 0:DK, blk * 4:(blk + 1) * 4].rearrange(
                        "p k j -> p j k"),
                    in_=v_ps)

                if blk < NBLK // 2:  # q-half lives in rows 0:2048 (rotated input)
                    q_ps = ps_q.tile([128, 512], F32)
                    for c in range(8):
                        nc.tensor.matmul(out=q_ps, lhsT=wq_sb[:, c, :],
                                         rhs=xt_blk[:, c, :, :],
                                         start=(c == 0), stop=(c == 7))
                    nc.vector.tensor_scalar_add(out=q_sb[:, sblk],
                                                in0=q_ps[0:DK, :], scalar1=bq_sb)

        # ---------------- phase 2: attention ----------------
        ot_sb = persist.tile([DK + 2, QC], F32R)  # O^T + sums row + pad row
        with tc.tile_pool(name="ap", bufs=3) as ap_pool, \
             tc.tile_pool(name="ps_sc", bufs=2, space="PSUM") as ps_sc, \
             tc.tile_pool(name="ps_o", bufs=1, space="PSUM") as ps_o:
            o_ps = [ps_o.tile([128, QB], F32, name=f"o_ps{i}")
                    for i in range(NQB)]
            for st in range(NT):
                for qb in range(NQB):
                    sc_ps = ps_sc.tile([128, QB], F32)
                    for j in range(QB // 512):
                        js = slice(j * 512, (j + 1) * 512)
                        nc.tensor.matmul(out=sc_ps[:, js],
                                         lhsT=k_sb[:, st * 128:(st + 1) * 128],
                                         rhs=q_sb[:, qb * QB + j * 512:
                                                  qb * QB + (j + 1) * 512],
                                         start=True, stop=True)
                    a_t = ap_pool.tile([128, QB], F32R)
                    nc.scalar.activation(out=a_t, in_=sc_ps,
                                         func=mybir.ActivationFunctionType.Exp,
                                         scale=0.125)
                    for j in range(QB // 512):
                        js = slice(j * 512, (j + 1) * 512)
                        nc.tensor.matmul(out=o_ps[qb][:, js],
                                         lhsT=vp_sb[:, :, st], rhs=a_t[:, js],
                                         start=(st == 0), stop=(st == NT - 1))
            for qb in range(NQB):
                nc.vector.tensor_copy(out=ot_sb[:, qb * QB:(qb + 1) * QB],
                                      in_=o_ps[qb][0:DK + 2, :])

        # ---------------- phase 3: normalize + store ----------------
        with tc.tile_pool(name="outp", bufs=3) as outp, \
             tc.tile_pool(name="smallp", bufs=3) as smallp, \
             tc.tile_pool(name="ps_on", bufs=2, space="PSUM") as ps_on:
            for qt in range(QC // 128):
                on_ps = ps_on.tile([128, DK + 2], F32R)
                nc.tensor.transpose(out=on_ps,
                                    in_=ot_sb[:, qt * 128:(qt + 1) * 128],
                                    identity=ident[0:DK + 2, 0:DK + 2])
                r_sb = smallp.tile([128, 1], F32)
                nc.vector.reciprocal(out=r_sb,
                                     in_=on_ps[:, DK:DK + 1].bitcast(F32))
                o_out = outp.tile([128, DK], F32)
                nc.vector.tensor_scalar_mul(out=o_out,
                                            in0=on_ps[:, 0:DK].bitcast(F32),
                                            scalar1=r_sb)
                nc.sync.dma_start(out=Y_d[qt * 128:(qt + 1) * 128, :], in_=o_out)


_NC_CACHE = []


def _get_nc():
    if not _NC_CACHE:
        nc = bass.Bass("TRN2", target_bir_lowering=False)
        with tile.TileContext(nc) as tc:
            _emit(tc)
        _install_waitsplit(nc)
        _NC_CACHE.append(nc)
    return _NC_CACHE[0]


def _pad_cols():
    pad = np.zeros((128, 128 - DK, NT), np.float32)
    pad[:, 0, :] = 1.0  # V' column 64: softmax-denominator ones
    return pad


def _common_inputs(WQ, bQ, WK, bK, WV, bV):
    return {
        "WQ": np.ascontiguousarray(np.asarray(WQ, np.float32)),
        "WK": np.ascontiguousarray(np.asarray(WK, np.float32)),
        "WV": np.ascontiguousarray(np.asarray(WV, np.float32)),
        "bQ": np.asarray(bQ, np.float32).reshape(DK, 1),
        "bK": np.asarray(bK, np.float32).reshape(DK, 1),
        "bV": np.asarray(bV, np.float32).reshape(DK, 1),
        "Ident": np.eye(128, dtype=np.float32),
        "PadCols": _pad_cols(),
    }


def kernel(X, cultural_embedding, WQ, bQ, WK, bK, WV, bV, WC, bC, lam):
    X = np.asarray(X, np.float32)
    common = _common_inputs(WQ, bQ, WK, bK, WV, bV)
    nc = _get_nc()
    in_maps = []
    for b in range(B):
        for h in range(2):
            Xb = X[b]
            if h == 1:
                Xb = np.concatenate([Xb[QC:], Xb[:QC]], axis=0)
            in_maps.append({"X": np.ascontiguousarray(Xb), **common})
    res = run_bass_kernel_spmd(nc, in_maps, list(range(NCORES)))
    out = np.empty((B, S, DK), np.float32)
    for b in range(B):
        for h in range(2):
            out[b, h * QC:(h + 1) * QC] = res.results[2 * b + h]["Y"]
    return out
